# revision 53
# baseline (speedup 1.0000x reference)
"""Causal self-attention (B=2, T=2048, C=1024, 16 heads x 64) on 8 TRN2 NeuronCores.

Sharding: core c -> batch b = c//4, heads [4*(c%4), 4*(c%4)+4)  (data parallel on B,
tensor parallel on heads). Each core computes QKV for its 4 heads, causal attention,
and a partial output projection (its 256 columns of y against W_proj). Host sums the
4 partials per batch and adds b_proj.

v3: the QKV projection runs as fp8e4m3 DoubleRow matmuls (2 contraction k-tiles per
instruction, 0.5 PE cycles/row) with error compensation: x ~ xh + xl and W*128 ~
Wh + Wl (each fp8), and the three cross terms xh*Wh + xl*Wh + xh*Wl accumulate in
PSUM (the lo*lo term is below bf16 noise).  The *128 pre-scale keeps W out of the
fp8 subnormal range; it is undone at the q/k fp8 staging (x1/128) and a /128 fold
into W_proj.

QK^T also runs in fp8 DoubleRow (0.5 cycles/row): q/k are staged to fp8 on DVE,
then a small SBUF->SBUF DMA folds each head's 64 dims into the DR layout
[32 partitions x 2-in-free] (dim d = 32*dp + di).  Per-head QK matmuls address
32-row PE tiles via explicit tile_position.  Pair-0 q/k is computed one 512-chunk
ahead of the attention chunk that needs it, so exp work starts ~10us into the
kernel instead of after the whole QKV.  Causal masking is a pre-exp additive
-1e9 on the PSUM scores (co-aligned tiles; diagonal-only adds).  exp outputs
attention weights P in bf16; V is bf16 (so the r==3 diagonal PV streams only its
live 128 columns); P@V accumulates f32 in PSUM with a ones-column producing the
softmax denominators.  Normalize = DVE reciprocal + a partition-0 shift DMA +
GPSIMD partition_broadcast (HW only broadcasts from/to partition 0) + DVE muls.
Partial outputs return in bf16.

Device layouts (host-prepped):
  xh8/xl8 (128, KD, 2, T) fp8: x[b].T hi/lo, rows split into KD double-tiles
  w[qkv][hl]8 (128, KD, 2, 256) fp8: W_qkv head-slices * 128, hi/lo, same split
  wp  (256, C) f32r : W_proj[:, head_cols].T / 128
  mka (128, 256) f32: additive causal mask (0 keep / -1e9 drop), right half is
      the diagonal triangle, left half all -1e9
"""

import ml_dtypes
import numpy as np

import concourse.bass as bass
import concourse.mybir as mybir
import concourse.tile as tile
from concourse import bacc
from concourse.bass_utils import run_bass_kernel_spmd

F32R = mybir.dt.float32r
F32 = mybir.dt.float32
BF16 = mybir.dt.bfloat16
F8 = mybir.dt.float8e4
EXP = mybir.ActivationFunctionType.Exp
DR = mybir.MatmulPerfMode.DoubleRow

B, T, C = 2, 2048, 1024
N_HEAD, HD = 16, 64
NHL = 4                 # heads per core
JD = NHL * HD           # 256 local q/k/v dims per core
TT = T // 128           # 16 row tiles
TC4 = T // 512          # 4 column chunks
WSC = 128.0             # fp8 pre-scale on W (keeps 0.02-std W out of subnormals)
SCALE = 1.0 / 8.0       # 1/sqrt(64); fp8 q/k staging already divides out WSC
N_CORES = 8


def build_program(qkv_bias: bool, reps: int = 1):
    KD = 5 if qkv_bias else 4           # 256-row double-tiles (5th folds bias)

    nc = bacc.Bacc("TRN2", target_bir_lowering=False, debug=False)
    xh_d = nc.dram_tensor("xh8", (128, KD, 2, T), F8, kind="ExternalInput")
    xl_d = nc.dram_tensor("xl8", (128, KD, 2, T), F8, kind="ExternalInput")
    w_d = {(w, p): nc.dram_tensor(f"w{w}{p}8", (128, KD, 2, JD), F8,
                                  kind="ExternalInput")
           for w in "qkv" for p in "hl"}
    wp_d = nc.dram_tensor("wp", (JD, C), F32R, kind="ExternalInput")
    mk_d = nc.dram_tensor("mka", (128, 256), F32, kind="ExternalInput")
    out_d = nc.dram_tensor("out", (T, C), BF16, kind="ExternalOutput")

    with (
        tile.TileContext(nc) as tc,
        tc.tile_pool(name="persist", bufs=1) as pp,
    ):
        # q/k in fp8 DoubleRow fold: head h -> partitions 32h..32h+32 (di),
        # free dims (dp, t) with dim d = 32*dp + di.  QK^T then runs as a
        # 0.5-cycle/row DR matmul per head (contraction 64 = 32 part x 2).
        qT8 = pp.tile([128, 2, T], F8, tag="qT8", name="qT8")
        kT8 = pp.tile([128, 2, T], F8, tag="kT8", name="kT8")
        # per-pair fp8 staging (lane layout, pre-fold): heads (2jt, 2jt+1)
        qst = [pp.tile([128, T], F8, tag=f"qst{j}", name=f"qst{j}")
               for j in range(2)]
        kst = [pp.tile([128, T], F8, tag=f"kst{j}", name=f"kst{j}")
               for j in range(2)]
        V = pp.tile([128, TT, 2, 193], BF16, tag="V")
        yTn = [pp.tile([128, T], F32R, tag=f"yTn{j}", name=f"yTn{j}") for j in range(2)]
        mka2 = pp.tile([128, 2, 256], F32, tag="mka2")

        psB = tc.alloc_tile_pool(name="psB", bufs=1, space="PSUM")
        ptp = tc.alloc_tile_pool(name="ptp", bufs=6)
        rcp = tc.alloc_tile_pool(name="rcp", bufs=4)
        wpp = tc.alloc_tile_pool(name="wpp", bufs=1)
        outp = tc.alloc_tile_pool(name="outp", bufs=6)
        xw = tc.alloc_tile_pool(name="xw", bufs=1)

        def body():
            nc.gpsimd.memset(V[:, :, :, 64:66], 1.0)   # ones columns
            nc.gpsimd.memset(V[:, :, :, 66:129], 0.0)  # odd zero pad
            w_sb = {key: xw.tile([128, KD, 2, JD], F8, name=f"w{key[0]}{key[1]}")
                    for key in w_d}
            xh_sb = xw.tile([128, KD, 2, T], F8, name="xh")
            xl_sb = xw.tile([128, KD, 2, T], F8, name="xl")
            # boot-critical DMA order: pair-0 t4=0 QKV inputs first so the
            # first attention chunk starts ~20us earlier; remaining x chunks
            # stream in behind the compute.
            for key in (("q", "h"), ("k", "h")):
                nc.sync.dma_start(w_sb[key][:], w_d[key].ap())
            nc.sync.dma_start(xh_sb[:, :, :, 0:512], xh_d.ap()[:, :, :, 0:512])
            for key in (("q", "l"), ("k", "l")):
                nc.sync.dma_start(w_sb[key][:], w_d[key].ap())
            nc.sync.dma_start(xl_sb[:, :, :, 0:512], xl_d.ap()[:, :, :, 0:512])
            for key in (("v", "h"), ("v", "l")):
                nc.sync.dma_start(w_sb[key][:], w_d[key].ap())
            for h2 in range(2):
                nc.sync.dma_start(mka2[:, h2, :], mk_d.ap())

            def fold_dmas(jt, t4=None):
                # qst/kst lane layout [64*hh + 32*dp + di, t] -> DR fold
                # [32*(2jt+hh) + di, dp, t]; contiguous 32-partition blocks.
                sl = slice(0, T) if t4 is None else slice(t4 * 512,
                                                          (t4 + 1) * 512)
                for st, dst in ((qst[jt], qT8), (kst[jt], kT8)):
                    for hh in range(2):
                        p0 = 32 * (2 * jt + hh)
                        for dp in range(2):
                            nc.sync.dma_start(
                                dst[p0:p0 + 32, dp, sl],
                                st[64 * hh + 32 * dp:64 * hh + 32 * dp + 32,
                                   sl])

            # the three compensation passes: xh*Wh + xh*Wl + xl*Wh
            # (xl consumed last so its DMA overlaps the first two passes)
            def passes(w):
                return ((xh_sb, w_sb[(w, "h")]), (xh_sb, w_sb[(w, "l")]),
                        (xl_sb, w_sb[(w, "h")]))

            def qk0_tile(t4):
                # pair-0 q/k for one 512-column chunk, staged + folded, just
                # ahead of the attention chunk that first needs it
                sl = slice(t4 * 512, (t4 + 1) * 512)
                bqk = psB.tile([128, 2, 512], F32, tag="sp", bufs=2,
                               name="bqk")
                bq, bk = bqk[:, 0, :], bqk[:, 1, :]
                for ps in range(3):
                    xs, wq_s = passes("q")[ps]
                    _, wk_s = passes("k")[ps]
                    for dt in range(KD):
                        first = ps == 0 and dt == 0
                        last = ps == 2 and dt == KD - 1
                        nc.tensor.matmul(
                            bq, wq_s[:, dt, :, 0:128], xs[:, dt, :, sl],
                            start=first, stop=last, perf_mode=DR)
                        nc.tensor.matmul(
                            bk, wk_s[:, dt, :, 0:128], xs[:, dt, :, sl],
                            start=first, stop=last, perf_mode=DR)
                with nc.allow_low_precision(reason="fp8 q/k for DR scores"):
                    nc.vector.tensor_scalar_mul(qst[0][:, sl], bq, 1.0 / WSC)
                    nc.vector.tensor_scalar_mul(kst[0][:, sl], bk, 1.0 / WSC)
                fold_dmas(0, t4)

            # v: (128 t x 256 d) tiles, split per head into V + ones columns
            def v_group(tts):
                for tt in tts:
                    vps = psB.tile([128, 512], F32, tag="stg", bufs=2, name="vps")
                    for ps in range(3):
                        xs, wv_s = passes("v")[ps]
                        for dt in range(KD):
                            nc.tensor.matmul(
                                vps[:, 0:JD],
                                xs[:, dt, :, tt * 128:(tt + 1) * 128],
                                wv_s[:, dt, :, :],
                                start=(ps == 0 and dt == 0),
                                stop=(ps == 2 and dt == KD - 1), perf_mode=DR,
                            )
                    # both head-pairs' even halves in one strided copy, then odd
                    with nc.allow_low_precision(reason="bf16 V"):
                        for half in range(2):
                            src = (vps[:, half * 64:half * 64 + 256]
                                   .rearrange("p (j d) -> p j d", j=2)[:, :, 0:64])
                            dst = (V[:, tt, :, 0:64] if half == 0
                                   else V[:, tt, :, 129:193])
                            nc.vector.tensor_copy(dst, src)

            def qk1_tiles(sel):
                items = [(w, dest, t4)
                         for t4 in range(4)
                         for w, dest in (("q", qst), ("k", kst))]
                for w, dest, t4 in items[sel]:
                    acc = psB.tile([128, 512], F32, tag="stg", bufs=2, name="acc")
                    for ps in range(3):
                        xs, w_s = passes(w)[ps]
                        for dt in range(KD):
                            nc.tensor.matmul(
                                acc[:],
                                w_s[:, dt, :, 128:256],
                                xs[:, dt, :, t4 * 512:(t4 + 1) * 512],
                                start=(ps == 0 and dt == 0),
                                stop=(ps == 2 and dt == KD - 1), perf_mode=DR,
                            )
                    with nc.allow_low_precision(reason="fp8 q/k for DR scores"):
                        nc.vector.tensor_scalar_mul(
                            dest[1][:, t4 * 512:(t4 + 1) * 512], acc[:],
                            1.0 / WSC)

            # ---- Attention + projection interleaved ----
            wp_sb = wpp.tile([128, 2, C], F32R)

            def attention_chunk(jt, j4, tgE, tgO):
                ypE = psB.tile([128, 512], F32, tag=tgE, bufs=2 if tgE == "stg" else 1,
                               name="ypE")
                ypO = psB.tile([128, 512], F32, tag=tgO, bufs=2 if tgO == "stg" else 1,
                               name="ypO")
                ni = 4 * j4 + 4
                D = 3                       # sT/exp emitted D tiles ahead of y
                order = list(range(ni))
                stash = []
                for s in range(ni + D):
                    if s < ni:
                        i = order[s]
                        r = i - 4 * j4
                        # co-aligned addressing: sp/pt col c holds q position
                        # j4*512 + c; live region is [co, 512)
                        w = 512 - 128 * max(r, 0)
                        co = 512 - w
                        sp = psB.tile([128, 2, 512], F32, tag="sp", bufs=2,
                                      name="sp")
                        for half in range(2):
                            p0 = 32 * (2 * jt + half)
                            nc.tensor.matmul(
                                sp[:, half, co:512],
                                kT8[p0:p0 + 32, :, i * 128:(i + 1) * 128],
                                qT8[p0:p0 + 32, :, j4 * 512 + co:(j4 + 1) * 512],
                                start=True, stop=True, perf_mode=DR,
                                tile_position=(p0, 0),
                            )
                        if r >= 0:
                            # additive causal mask on the diagonal 128 columns
                            nc.vector.tensor_add(
                                sp[:, :, co:co + 128], sp[:, :, co:co + 128],
                                mka2[:, :, 128:256])
                        pt = ptp.tile([128, 2, 512], BF16, tag="pt", name="pt")
                        with nc.allow_low_precision(reason="bf16 attn weights"):
                            nc.scalar.activation(pt[:, :, co:512],
                                                 sp[:, :, co:512], EXP,
                                                 scale=SCALE)
                        stash.append((pt, co, i, s))
                    if s >= D:
                        pt, co, i, si = stash.pop(0)
                        cp = co
                        for half in range(2):
                            yp = ypE[0:65] if half == 0 else ypO[:]
                            vs = (V[:, i, jt, 0:65] if half == 0
                                  else V[:, i, jt, 65:193])
                            nc.tensor.matmul(
                                yp[:, cp:512], vs, pt[:, half, cp:512],
                                start=(si == 0), stop=(si == ni - 1),
                            )
                # normalize: yTn = y * bcast(1/denom) -- reciprocal on DVE,
                # broadcast over partitions on the (idle) Pool engine, then
                # one DVE mul per half with ypE/ypO as the single PSUM operand.
                # The reciprocals issue now (DVE runs them as soon as the PV
                # accumulation stops); broadcast + muls return as a closure so
                # the caller can slot independent work first.
                ts4 = slice(j4 * 512, (j4 + 1) * 512)
                recE = rcp.tile([128, 512], F32R, tag="recE", name="recE")
                recO = rcp.tile([128, 512], F32R, tag="recO", name="recO")
                with nc.allow_low_precision(reason="float32r is fp32 storage"):
                    nc.vector.reciprocal(recE[64:65, :], ypE[64:65, :])
                    nc.vector.reciprocal(recO[0:1, :], ypO[0:1, :])
                # HW partition_broadcast only reads partition 0: DMA-shift the
                # E-half reciprocal row from partition 64 down to partition 0
                nc.gpsimd.dma_start(recE[0:1, :], recE[64:65, :])

                bctE = rcp.tile([128, 512], F32R, tag="yE", name="bctE")
                bctO = rcp.tile([128, 512], F32R, tag="yO", name="bctO")
                # HW partition_broadcast needs base partition 0 on BOTH
                # operands; write all 128 partitions (cost is free-size)
                nc.gpsimd.partition_broadcast(bctE[:, :], recE[0:1, :])
                nc.gpsimd.partition_broadcast(bctO[:, :], recO[0:1, :])

                def finish():
                    nc.vector.tensor_mul(yTn[jt][0:64, ts4], ypE[0:64, :],
                                         bctE[0:64, :])
                    nc.vector.tensor_mul(yTn[jt][64:128, ts4], ypO[64:128, :],
                                         bctO[64:128, :])
                return finish

            def proj_block(j4, ptags):
                for tt in range(4 * j4, 4 * j4 + 4):
                    ob = outp.tile([128, C], BF16, tag="ob", name="ob")
                    for nh in range(2):
                        ops = psB.tile([128, 512], F32, tag=ptags[nh],
                                       bufs=2 if ptags[nh] == "stg" else 1,
                                       name="ops")
                        for kc in range(2):
                            nc.tensor.matmul(
                                ops[:],
                                yTn[kc][:, tt * 128:(tt + 1) * 128],
                                wp_sb[:, kc, nh * 512:(nh + 1) * 512],
                                start=(kc == 0), stop=(kc == 1),
                            )
                        with nc.allow_low_precision(reason="bf16 partial out"):
                            nc.vector.tensor_copy(
                                ob[:, nh * 512:(nh + 1) * 512], ops[:])
                    nc.sync.dma_start(out_d.ap()[tt * 128:(tt + 1) * 128, :], ob[:])

            # head pair 0: attention interleaved between v-tile groups so the
            # ACT engine starts exp-ing while the PE is still on projections;
            # each chunk's normalize lands after qk1's independent PE work
            qk0_tile(0)
            for j4 in range(TC4):
                if j4 + 1 < TC4:
                    # stream the next x chunk early; its QKV issues after
                    # the v tiles
                    sl = slice((j4 + 1) * 512, (j4 + 2) * 512)
                    nc.sync.dma_start(xh_sb[:, :, :, sl],
                                      xh_d.ap()[:, :, :, sl])
                    nc.sync.dma_start(xl_sb[:, :, :, sl],
                                      xl_d.ap()[:, :, :, sl])
                v_group(range(4 * j4, 4 * j4 + 4))
                if j4 + 1 < TC4:
                    qk0_tile(j4 + 1)     # prefetch next chunk's q/k
                fin = attention_chunk(0, j4, "ypE", "ypO")
                qk1_tiles(slice(2 * j4, 2 * j4 + 2))
                fin()
            fold_dmas(1)
            for kc in range(2):
                nc.sync.dma_start(wp_sb[:, kc, :],
                                  wp_d.ap()[128 * kc:128 * (kc + 1), :])
            # head pair 1 largest-chunk-first (short drain tail), parity tags
            for n, j4 in enumerate((3, 2, 1, 0)):
                tags = ("stg", "stg") if n % 2 == 0 else ("ypE", "ypO")
                fin = attention_chunk(1, j4, *tags)
                fin()
                proj_block(j4, tags)

        if reps > 1:
            with tc.For_i(0, reps, 1):
                body()
        else:
            body()
        xw.release()
        outp.release()
        wpp.release()
        rcp.release()
        ptp.release()
        psB.release()

    nc.compile()
    return nc


_PROGRAM_CACHE = {}


def get_program(qkv_bias: bool, reps: int = 1):
    key = (qkv_bias, reps)
    if key not in _PROGRAM_CACHE:
        _PROGRAM_CACHE[key] = build_program(qkv_bias, reps)
    return _PROGRAM_CACHE[key]


def make_in_maps(x, W_qkv, b_qkv, W_proj):
    qkv_bias = bool(np.any(b_qkv != 0.0))
    KD = 5 if qkv_bias else 4
    KR = KD * 256
    u = np.arange(256)[None, :]
    mka = np.where(np.arange(128)[:, None] <= u - 128, 0.0,
                   -1e9).astype(np.float32)
    f8 = ml_dtypes.float8_e4m3

    def to_dr(a, row_c, ncol):
        """(rows, ncol) f32 -> fp8 hi/lo pair in (128, KD, 2, ncol) layout."""
        full = np.zeros((KR, ncol), np.float32)
        full[:a.shape[0]] = a
        if row_c is not None:
            full[C] = row_c
        hi = full.astype(f8)
        lo = (full - hi.astype(np.float32)).astype(f8)
        dr = lambda m: np.ascontiguousarray(
            m.reshape(KD, 2, 128, ncol).transpose(2, 0, 1, 3))
        return dr(hi), dr(lo)

    in_maps = []
    for c in range(N_CORES):
        b, hg = c // 4, c % 4
        r0 = hg * JD
        xT = np.ascontiguousarray(x[b].T)                      # (C, T)
        xh, xl = to_dr(xT, np.ones(T, np.float32) if qkv_bias else None, T)
        im = {"xh8": xh, "xl8": xl,
              "wp": np.ascontiguousarray(W_proj[:, r0:r0 + JD].T) / WSC,
              "mka": mka}
        for w, rbase in (("q", 0), ("k", C), ("v", 2 * C)):
            ws = np.ascontiguousarray(W_qkv[rbase + r0:rbase + r0 + JD, :].T) * WSC
            bias = b_qkv[rbase + r0:rbase + r0 + JD] * WSC if qkv_bias else None
            hi, lo = to_dr(ws, bias, JD)
            im[f"w{w}h8"], im[f"w{w}l8"] = hi, lo
        in_maps.append(im)
    return in_maps, qkv_bias


def kernel(x, W_qkv, b_qkv, W_proj, b_proj):
    x = np.asarray(x, dtype=np.float32)
    W_qkv = np.asarray(W_qkv, dtype=np.float32)
    b_qkv = np.asarray(b_qkv, dtype=np.float32)
    W_proj = np.asarray(W_proj, dtype=np.float32)
    b_proj = np.asarray(b_proj, dtype=np.float32)

    in_maps, qkv_bias = make_in_maps(x, W_qkv, b_qkv, W_proj)
    nc = get_program(qkv_bias)
    results = run_bass_kernel_spmd(nc, in_maps, core_ids=list(range(N_CORES))).results

    out = np.empty((B, T, C), dtype=np.float32)
    for b in range(B):
        acc = results[4 * b]["out"].astype(np.float32)
        for hg in range(1, 4):
            acc += results[4 * b + hg]["out"].astype(np.float32)
        out[b] = acc + b_proj[None, :]
    return out



# revision 54
# speedup vs baseline: 1.0002x; 1.0002x over previous
"""Causal self-attention (B=2, T=2048, C=1024, 16 heads x 64) on 8 TRN2 NeuronCores.

Sharding: core c -> batch b = c//4, heads [4*(c%4), 4*(c%4)+4)  (data parallel on B,
tensor parallel on heads). Each core computes QKV for its 4 heads, causal attention,
and a partial output projection (its 256 columns of y against W_proj). Host sums the
4 partials per batch and adds b_proj.

v3: the QKV projection runs as fp8e4m3 DoubleRow matmuls (2 contraction k-tiles per
instruction, 0.5 PE cycles/row) with error compensation: x ~ xh + xl and W*128 ~
Wh + Wl (each fp8), and the three cross terms xh*Wh + xl*Wh + xh*Wl accumulate in
PSUM (the lo*lo term is below bf16 noise).  The *128 pre-scale keeps W out of the
fp8 subnormal range; it is undone at the q/k fp8 staging (x1/128) and a /128 fold
into W_proj.

QK^T also runs in fp8 DoubleRow (0.5 cycles/row): q/k are staged to fp8 on DVE,
then a small SBUF->SBUF DMA folds each head's 64 dims into the DR layout
[32 partitions x 2-in-free] (dim d = 32*dp + di).  Per-head QK matmuls address
32-row PE tiles via explicit tile_position.  Pair-0 q/k is computed one 512-chunk
ahead of the attention chunk that needs it, so exp work starts ~10us into the
kernel instead of after the whole QKV.  Causal masking is a pre-exp additive
-1e9 on the PSUM scores (co-aligned tiles; diagonal-only adds).  exp outputs
attention weights P in bf16; V is bf16 (so the r==3 diagonal PV streams only its
live 128 columns); P@V accumulates f32 in PSUM with a ones-column producing the
softmax denominators.  Normalize = DVE reciprocal + a partition-0 shift DMA +
GPSIMD partition_broadcast (HW only broadcasts from/to partition 0) + DVE muls.
Partial outputs return in bf16.

Device layouts (host-prepped):
  xh8/xl8 (128, KD, 2, T) fp8: x[b].T hi/lo, rows split into KD double-tiles
  w[qkv][hl]8 (128, KD, 2, 256) fp8: W_qkv head-slices * 128, hi/lo, same split
  wp  (256, C) f32r : W_proj[:, head_cols].T / 128
  mka (128, 256) f32: additive causal mask (0 keep / -1e9 drop), right half is
      the diagonal triangle, left half all -1e9
"""

import ml_dtypes
import numpy as np

import concourse.bass as bass
import concourse.mybir as mybir
import concourse.tile as tile
from concourse import bacc
from concourse.bass_utils import run_bass_kernel_spmd

F32R = mybir.dt.float32r
F32 = mybir.dt.float32
BF16 = mybir.dt.bfloat16
F8 = mybir.dt.float8e4
EXP = mybir.ActivationFunctionType.Exp
DR = mybir.MatmulPerfMode.DoubleRow

B, T, C = 2, 2048, 1024
N_HEAD, HD = 16, 64
NHL = 4                 # heads per core
JD = NHL * HD           # 256 local q/k/v dims per core
TT = T // 128           # 16 row tiles
TC4 = T // 512          # 4 column chunks
WSC = 128.0             # fp8 pre-scale on W (keeps 0.02-std W out of subnormals)
SCALE = 1.0 / 8.0       # 1/sqrt(64); fp8 q/k staging already divides out WSC
N_CORES = 8


def build_program(qkv_bias: bool, reps: int = 1):
    KD = 5 if qkv_bias else 4           # 256-row double-tiles (5th folds bias)

    nc = bacc.Bacc("TRN2", target_bir_lowering=False, debug=False)
    xh_d = nc.dram_tensor("xh8", (128, KD, 2, T), F8, kind="ExternalInput")
    xl_d = nc.dram_tensor("xl8", (128, KD, 2, T), F8, kind="ExternalInput")
    w_d = {(w, p): nc.dram_tensor(f"w{w}{p}8", (128, KD, 2, JD), F8,
                                  kind="ExternalInput")
           for w in "qkv" for p in "hl"}
    wp_d = nc.dram_tensor("wp", (JD, C), F32R, kind="ExternalInput")
    mk_d = nc.dram_tensor("mka", (128, 256), F32, kind="ExternalInput")
    out_d = nc.dram_tensor("out", (T, C), BF16, kind="ExternalOutput")

    with (
        tile.TileContext(nc) as tc,
        tc.tile_pool(name="persist", bufs=1) as pp,
    ):
        # q/k in fp8 DoubleRow fold: head h -> partitions 32h..32h+32 (di),
        # free dims (dp, t) with dim d = 32*dp + di.  QK^T then runs as a
        # 0.5-cycle/row DR matmul per head (contraction 64 = 32 part x 2).
        qT8 = pp.tile([128, 2, T], F8, tag="qT8", name="qT8")
        kT8 = pp.tile([128, 2, T], F8, tag="kT8", name="kT8")
        # per-pair fp8 staging (lane layout, pre-fold): heads (2jt, 2jt+1)
        qst = [pp.tile([128, T], F8, tag=f"qst{j}", name=f"qst{j}")
               for j in range(2)]
        kst = [pp.tile([128, T], F8, tag=f"kst{j}", name=f"kst{j}")
               for j in range(2)]
        V = pp.tile([128, TT, 2, 193], BF16, tag="V")
        yTn = [pp.tile([128, T], F32R, tag=f"yTn{j}", name=f"yTn{j}") for j in range(2)]
        mka2 = pp.tile([128, 2, 256], F32, tag="mka2")

        psB = tc.alloc_tile_pool(name="psB", bufs=1, space="PSUM")
        ptp = tc.alloc_tile_pool(name="ptp", bufs=6)
        rcp = tc.alloc_tile_pool(name="rcp", bufs=4)
        wpp = tc.alloc_tile_pool(name="wpp", bufs=1)
        outp = tc.alloc_tile_pool(name="outp", bufs=6)
        xw = tc.alloc_tile_pool(name="xw", bufs=1)

        def body():
            nc.gpsimd.memset(V[:, :, :, 64:66], 1.0)   # ones columns
            nc.gpsimd.memset(V[:, :, :, 66:129], 0.0)  # odd zero pad
            w_sb = {key: xw.tile([128, KD, 2, JD], F8, name=f"w{key[0]}{key[1]}")
                    for key in w_d}
            xh_sb = xw.tile([128, KD, 2, T], F8, name="xh")
            xl_sb = xw.tile([128, KD, 2, T], F8, name="xl")
            # boot-critical DMA order: pair-0 t4=0 QKV inputs first so the
            # first attention chunk starts ~20us earlier; remaining x chunks
            # stream in behind the compute.
            for key in (("q", "h"), ("k", "h")):
                nc.sync.dma_start(w_sb[key][:], w_d[key].ap())
            nc.sync.dma_start(xh_sb[:, :, :, 0:512], xh_d.ap()[:, :, :, 0:512])
            for key in (("q", "l"), ("k", "l")):
                nc.sync.dma_start(w_sb[key][:], w_d[key].ap())
            nc.sync.dma_start(xl_sb[:, :, :, 0:512], xl_d.ap()[:, :, :, 0:512])
            for key in (("v", "h"), ("v", "l")):
                nc.sync.dma_start(w_sb[key][:], w_d[key].ap())
            for h2 in range(2):
                nc.sync.dma_start(mka2[:, h2, :], mk_d.ap())

            def fold_dmas(jt, t4=None):
                # qst/kst lane layout [64*hh + 32*dp + di, t] -> DR fold
                # [32*(2jt+hh) + di, dp, t]; contiguous 32-partition blocks.
                sl = slice(0, T) if t4 is None else slice(t4 * 512,
                                                          (t4 + 1) * 512)
                for st, dst in ((qst[jt], qT8), (kst[jt], kT8)):
                    for hh in range(2):
                        p0 = 32 * (2 * jt + hh)
                        for dp in range(2):
                            nc.sync.dma_start(
                                dst[p0:p0 + 32, dp, sl],
                                st[64 * hh + 32 * dp:64 * hh + 32 * dp + 32,
                                   sl])

            # the three compensation passes: xh*Wh + xh*Wl + xl*Wh
            # (xl consumed last so its DMA overlaps the first two passes)
            def passes(w):
                return ((xh_sb, w_sb[(w, "h")]), (xh_sb, w_sb[(w, "l")]),
                        (xl_sb, w_sb[(w, "h")]))

            def qk0_tile(t4):
                # pair-0 q/k for one 512-column chunk, staged + folded, just
                # ahead of the attention chunk that first needs it
                sl = slice(t4 * 512, (t4 + 1) * 512)
                bqk = psB.tile([128, 2, 512], F32, tag="sp", bufs=2,
                               name="bqk")
                bq, bk = bqk[:, 0, :], bqk[:, 1, :]
                for ps in range(3):
                    xs, wq_s = passes("q")[ps]
                    _, wk_s = passes("k")[ps]
                    for dt in range(KD):
                        first = ps == 0 and dt == 0
                        last = ps == 2 and dt == KD - 1
                        nc.tensor.matmul(
                            bq, wq_s[:, dt, :, 0:128], xs[:, dt, :, sl],
                            start=first, stop=last, perf_mode=DR)
                        nc.tensor.matmul(
                            bk, wk_s[:, dt, :, 0:128], xs[:, dt, :, sl],
                            start=first, stop=last, perf_mode=DR)
                with nc.allow_low_precision(reason="fp8 q/k for DR scores"):
                    nc.vector.tensor_scalar_mul(qst[0][:, sl], bq, 1.0 / WSC)
                    nc.vector.tensor_scalar_mul(kst[0][:, sl], bk, 1.0 / WSC)
                fold_dmas(0, t4)

            # v: (128 t x 256 d) tiles, split per head into V + ones columns
            def v_group(tts):
                for tt in tts:
                    vps = psB.tile([128, 512], F32, tag="stg", bufs=2, name="vps")
                    for ps in range(3):
                        xs, wv_s = passes("v")[ps]
                        for dt in range(KD):
                            nc.tensor.matmul(
                                vps[:, 0:JD],
                                xs[:, dt, :, tt * 128:(tt + 1) * 128],
                                wv_s[:, dt, :, :],
                                start=(ps == 0 and dt == 0),
                                stop=(ps == 2 and dt == KD - 1), perf_mode=DR,
                            )
                    # both head-pairs' even halves in one strided copy, then odd
                    with nc.allow_low_precision(reason="bf16 V"):
                        for half in range(2):
                            src = (vps[:, half * 64:half * 64 + 256]
                                   .rearrange("p (j d) -> p j d", j=2)[:, :, 0:64])
                            dst = (V[:, tt, :, 0:64] if half == 0
                                   else V[:, tt, :, 129:193])
                            nc.vector.tensor_copy(dst, src)

            def qk1_tiles(sel):
                items = [(w, dest, t4)
                         for t4 in range(4)
                         for w, dest in (("q", qst), ("k", kst))]
                for w, dest, t4 in items[sel]:
                    acc = psB.tile([128, 512], F32, tag="stg", bufs=2, name="acc")
                    for ps in range(3):
                        xs, w_s = passes(w)[ps]
                        for dt in range(KD):
                            nc.tensor.matmul(
                                acc[:],
                                w_s[:, dt, :, 128:256],
                                xs[:, dt, :, t4 * 512:(t4 + 1) * 512],
                                start=(ps == 0 and dt == 0),
                                stop=(ps == 2 and dt == KD - 1), perf_mode=DR,
                            )
                    with nc.allow_low_precision(reason="fp8 q/k for DR scores"):
                        nc.vector.tensor_scalar_mul(
                            dest[1][:, t4 * 512:(t4 + 1) * 512], acc[:],
                            1.0 / WSC)

            # ---- Attention + projection interleaved ----
            wp_sb = wpp.tile([128, 2, C], F32R)

            def attention_chunk(jt, j4, tgE, tgO):
                ypE = psB.tile([128, 512], F32, tag=tgE, bufs=2 if tgE == "stg" else 1,
                               name="ypE")
                ypO = psB.tile([128, 512], F32, tag=tgO, bufs=2 if tgO == "stg" else 1,
                               name="ypO")
                ni = 4 * j4 + 4
                D = 3                       # sT/exp emitted D tiles ahead of y
                order = list(range(ni))
                stash = []
                for s in range(ni + D):
                    if s < ni:
                        i = order[s]
                        r = i - 4 * j4
                        # co-aligned addressing: sp/pt col c holds q position
                        # j4*512 + c; live region is [co, 512)
                        w = 512 - 128 * max(r, 0)
                        co = 512 - w
                        sp = psB.tile([128, 2, 512], F32, tag="sp", bufs=2,
                                      name="sp")
                        for half in range(2):
                            p0 = 32 * (2 * jt + half)
                            nc.tensor.matmul(
                                sp[:, half, co:512],
                                kT8[p0:p0 + 32, :, i * 128:(i + 1) * 128],
                                qT8[p0:p0 + 32, :, j4 * 512 + co:(j4 + 1) * 512],
                                start=True, stop=True, perf_mode=DR,
                                tile_position=(p0, 0),
                            )
                        if r >= 0:
                            # additive causal mask on the diagonal 128 columns
                            nc.vector.tensor_add(
                                sp[:, :, co:co + 128], sp[:, :, co:co + 128],
                                mka2[:, :, 128:256])
                        pt = ptp.tile([128, 2, 512], BF16, tag="pt", name="pt")
                        with nc.allow_low_precision(reason="bf16 attn weights"):
                            nc.scalar.activation(pt[:, :, co:512],
                                                 sp[:, :, co:512], EXP,
                                                 scale=SCALE)
                        stash.append((pt, co, i, s))
                    if s >= D:
                        pt, co, i, si = stash.pop(0)
                        cp = co
                        for half in range(2):
                            yp = ypE[0:65] if half == 0 else ypO[:]
                            vs = (V[:, i, jt, 0:65] if half == 0
                                  else V[:, i, jt, 65:193])
                            nc.tensor.matmul(
                                yp[:, cp:512], vs, pt[:, half, cp:512],
                                start=(si == 0), stop=(si == ni - 1),
                            )
                # normalize: yTn = y * bcast(1/denom) -- reciprocal on DVE,
                # broadcast over partitions on the (idle) Pool engine, then
                # one DVE mul per half with ypE/ypO as the single PSUM operand.
                # The reciprocals issue now (DVE runs them as soon as the PV
                # accumulation stops); broadcast + muls return as a closure so
                # the caller can slot independent work first.
                ts4 = slice(j4 * 512, (j4 + 1) * 512)
                recE = rcp.tile([128, 512], F32R, tag="recE", name="recE")
                recO = rcp.tile([128, 512], F32R, tag="recO", name="recO")
                with nc.allow_low_precision(reason="float32r is fp32 storage"):
                    nc.vector.reciprocal(recE[64:65, :], ypE[64:65, :])
                    nc.vector.reciprocal(recO[0:1, :], ypO[0:1, :])
                # HW partition_broadcast only reads partition 0: DMA-shift the
                # E-half reciprocal row from partition 64 down to partition 0
                nc.gpsimd.dma_start(recE[0:1, :], recE[64:65, :])

                bctE = rcp.tile([128, 512], F32R, tag="yE", name="bctE")
                bctO = rcp.tile([128, 512], F32R, tag="yO", name="bctO")
                # HW partition_broadcast needs base partition 0 on BOTH
                # operands; write all 128 partitions (cost is free-size)
                nc.gpsimd.partition_broadcast(bctE[:, :], recE[0:1, :])
                nc.gpsimd.partition_broadcast(bctO[:, :], recO[0:1, :])

                def finish():
                    nc.vector.tensor_mul(yTn[jt][0:64, ts4], ypE[0:64, :],
                                         bctE[0:64, :])
                    nc.vector.tensor_mul(yTn[jt][64:128, ts4], ypO[64:128, :],
                                         bctO[64:128, :])
                return finish

            def proj_block(j4, ptags):
                for tt in range(4 * j4, 4 * j4 + 4):
                    ob = outp.tile([128, C], BF16, tag="ob", name="ob")
                    for nh in range(2):
                        ops = psB.tile([128, 512], F32, tag=ptags[nh],
                                       bufs=2 if ptags[nh] == "stg" else 1,
                                       name="ops")
                        for kc in range(2):
                            nc.tensor.matmul(
                                ops[:],
                                yTn[kc][:, tt * 128:(tt + 1) * 128],
                                wp_sb[:, kc, nh * 512:(nh + 1) * 512],
                                start=(kc == 0), stop=(kc == 1),
                            )
                        with nc.allow_low_precision(reason="bf16 partial out"):
                            nc.vector.tensor_copy(
                                ob[:, nh * 512:(nh + 1) * 512], ops[:])
                    nc.sync.dma_start(out_d.ap()[tt * 128:(tt + 1) * 128, :], ob[:])

            # head pair 0: attention interleaved between v-tile groups so the
            # ACT engine starts exp-ing while the PE is still on projections;
            # each chunk's normalize lands after qk1's independent PE work
            qk0_tile(0)
            for j4 in range(TC4):
                if j4 + 1 < TC4:
                    # stream the next x chunk early; its QKV issues after
                    # the v tiles
                    sl = slice((j4 + 1) * 512, (j4 + 2) * 512)
                    nc.sync.dma_start(xh_sb[:, :, :, sl],
                                      xh_d.ap()[:, :, :, sl])
                    nc.sync.dma_start(xl_sb[:, :, :, sl],
                                      xl_d.ap()[:, :, :, sl])
                v_group(range(4 * j4, 4 * j4 + 4))
                if j4 + 1 < TC4:
                    qk0_tile(j4 + 1)     # prefetch next chunk's q/k
                fin = attention_chunk(0, j4, "ypE", "ypO")
                qk1_tiles(slice(2 * j4, 2 * j4 + 2))
                fin()
            fold_dmas(1)
            for kc in range(2):
                nc.sync.dma_start(wp_sb[:, kc, :],
                                  wp_d.ap()[128 * kc:128 * (kc + 1), :])
            # head pair 1 largest-chunk-first (short drain tail), parity tags
            for n, j4 in enumerate((3, 2, 1, 0)):
                tags = ("ypE", "ypO") if n % 2 == 0 else ("stg", "stg")
                fin = attention_chunk(1, j4, *tags)
                fin()
                proj_block(j4, tags)

        if reps > 1:
            with tc.For_i(0, reps, 1):
                body()
        else:
            body()
        xw.release()
        outp.release()
        wpp.release()
        rcp.release()
        ptp.release()
        psB.release()

    nc.compile()
    return nc


_PROGRAM_CACHE = {}


def get_program(qkv_bias: bool, reps: int = 1):
    key = (qkv_bias, reps)
    if key not in _PROGRAM_CACHE:
        _PROGRAM_CACHE[key] = build_program(qkv_bias, reps)
    return _PROGRAM_CACHE[key]


def make_in_maps(x, W_qkv, b_qkv, W_proj):
    qkv_bias = bool(np.any(b_qkv != 0.0))
    KD = 5 if qkv_bias else 4
    KR = KD * 256
    u = np.arange(256)[None, :]
    mka = np.where(np.arange(128)[:, None] <= u - 128, 0.0,
                   -1e9).astype(np.float32)
    f8 = ml_dtypes.float8_e4m3

    def to_dr(a, row_c, ncol):
        """(rows, ncol) f32 -> fp8 hi/lo pair in (128, KD, 2, ncol) layout."""
        full = np.zeros((KR, ncol), np.float32)
        full[:a.shape[0]] = a
        if row_c is not None:
            full[C] = row_c
        hi = full.astype(f8)
        lo = (full - hi.astype(np.float32)).astype(f8)
        dr = lambda m: np.ascontiguousarray(
            m.reshape(KD, 2, 128, ncol).transpose(2, 0, 1, 3))
        return dr(hi), dr(lo)

    in_maps = []
    for c in range(N_CORES):
        b, hg = c // 4, c % 4
        r0 = hg * JD
        xT = np.ascontiguousarray(x[b].T)                      # (C, T)
        xh, xl = to_dr(xT, np.ones(T, np.float32) if qkv_bias else None, T)
        im = {"xh8": xh, "xl8": xl,
              "wp": np.ascontiguousarray(W_proj[:, r0:r0 + JD].T) / WSC,
              "mka": mka}
        for w, rbase in (("q", 0), ("k", C), ("v", 2 * C)):
            ws = np.ascontiguousarray(W_qkv[rbase + r0:rbase + r0 + JD, :].T) * WSC
            bias = b_qkv[rbase + r0:rbase + r0 + JD] * WSC if qkv_bias else None
            hi, lo = to_dr(ws, bias, JD)
            im[f"w{w}h8"], im[f"w{w}l8"] = hi, lo
        in_maps.append(im)
    return in_maps, qkv_bias


def kernel(x, W_qkv, b_qkv, W_proj, b_proj):
    x = np.asarray(x, dtype=np.float32)
    W_qkv = np.asarray(W_qkv, dtype=np.float32)
    b_qkv = np.asarray(b_qkv, dtype=np.float32)
    W_proj = np.asarray(W_proj, dtype=np.float32)
    b_proj = np.asarray(b_proj, dtype=np.float32)

    in_maps, qkv_bias = make_in_maps(x, W_qkv, b_qkv, W_proj)
    nc = get_program(qkv_bias)
    results = run_bass_kernel_spmd(nc, in_maps, core_ids=list(range(N_CORES))).results

    out = np.empty((B, T, C), dtype=np.float32)
    for b in range(B):
        acc = results[4 * b]["out"].astype(np.float32)
        for hg in range(1, 4):
            acc += results[4 * b + hg]["out"].astype(np.float32)
        out[b] = acc + b_proj[None, :]
    return out

